# revision 9
# baseline (speedup 1.0000x reference)
"""Distributed attention kernel for 8 TRN2 NeuronCores.

Problem: L=2048, B=2, E=256, H=8 heads, D=32 head-dim, fp32.

Sharding: DP2 over batch x sequence-parallel-4 over query positions.
Core c handles batch c//4, query rows [512*(c%4), 512*(c%4+1)), ALL 8
heads. k/v projections are redundantly computed per batch group (cheap)
and NO collective is needed: each core owns a disjoint output block.

Per-core pipeline (v2 -- row-tiled scores + dual-engine exp):
  - kT/qT are stored with 4 heads stacked on partition bands 0/32/64/96
    so the K=32 score matmuls for 4 heads run CONCURRENTLY in the PE
    array via tile_position=(32u, 0) row tiling (the 128x128 array is
    16 32x32 subarrays; row-tiled matmuls with disjoint row groups
    overlap almost perfectly).
  - softmax exp is split across TWO engines running in parallel:
    ScalarE does exact exp via the ACT LUT; VectorE (DVE) computes a
    Schraudolph-style exp2 approximation (one fused mult+add
    tensor_scalar producing the bf16 BIT PATTERN as int16). The
    approximation has ~1.8% rms relative error, softmax-normalized;
    only a minority of key-blocks use it so the output error stays
    ~1.3% (budget 2e-2).
  - k/v projections are interleaved into pass 0's attention loop so PE
    never idles; pass 1 finalizes per-tq-half to shorten the tail.
  - PV uses P.T chunks as the STATIONARY operand and [v|1] as moving,
    so O lands in natural [tq, d] orientation with the softmax
    denominator Z per-partition (reciprocal + broadcast multiply);
    xbar DMA transposes produce O.T for the projection off-PE.
"""

import math
import os
import sys

import numpy as np

for _p in ("/opt/trn_rl_repo",):
    if _p not in sys.path and os.path.isdir(_p):
        sys.path.insert(0, _p)

import ml_dtypes

import concourse.bass as bass
import concourse.bacc as bacc
import concourse.mybir as mybir
import concourse.tile as tile
from concourse.bass_utils import run_bass_kernel_spmd

dt = mybir.dt
F32 = dt.float32
BF16 = dt.bfloat16
I16 = dt.int16
AF = mybir.ActivationFunctionType
ALU = mybir.AluOpType
BF = ml_dtypes.bfloat16

L, B, E, H, D = 2048, 2, 256, 8, 32
SCALE = float(D) ** -0.5
NCORES = 8
SP = 4            # sequence-parallel ways
TQ = L // SP      # 512 query rows per core
NTK = L // 128    # 16 tk chunks
VW = H * (D + 1)  # v_buf cols per tk chunk: 8x [v_h | 1] = 264
NPASS = 2         # head passes (4 heads each)

# Schraudolph exp2: bf16 bits of exp(s*SCALE) ~= int16(s*C1 + C2).
# C1 = 128 * SCALE * log2(e); C2 = 128*(127 - c) + 0.5 (c = rms-optimal
# 0.0434, +0.5 assumes truncating convert; the c offset is uniform
# across a softmax row so only the intra-octave wiggle matters).
C1 = 128.0 * SCALE * math.log2(math.e)
C2 = 128.0 * (127.0 - 0.0434) + 0.5

# exp engine assignment (True -> DVE Schraudolph, False -> ScalarE exp).
# pass 0 has 8 units per n-group (idx = (g%4)*2 + j); pass 1 has 32
# units (idx = j*16 + g).
DVE_P0 = (1, 4, 6)
DVE_P1 = tuple(i for i in range(32) if i % 2 == 1 and i not in (1, 17))

_GRAPH = None


def _build_graph():
    nc = bacc.Bacc(
        "TRN2",
        target_bir_lowering=False,
        debug=False,
        enable_asserts=False,
        num_devices=NCORES,
    )

    xqt = nc.declare_dram_parameter("xqt", [E, TQ], BF16, isOutput=False).ap()
    xkt = nc.declare_dram_parameter("xkt", [E, L], BF16, isOutput=False).ap()
    xvt = nc.declare_dram_parameter("xvt", [E, L], BF16, isOutput=False).ap()
    wq = nc.declare_dram_parameter("wq", [E, E], BF16, isOutput=False).ap()
    wk = nc.declare_dram_parameter("wk", [E, E], BF16, isOutput=False).ap()
    wv = nc.declare_dram_parameter("wv", [E, E], BF16, isOutput=False).ap()
    wp = nc.declare_dram_parameter("wp", [E, E], BF16, isOutput=False).ap()
    bq = nc.declare_dram_parameter("bq", [1, E], F32, isOutput=False).ap()
    bk = nc.declare_dram_parameter("bk", [1, E], F32, isOutput=False).ap()
    bv = nc.declare_dram_parameter("bv", [1, E], F32, isOutput=False).ap()
    bp = nc.declare_dram_parameter("bp", [1, E], F32, isOutput=False).ap()
    out = nc.declare_dram_parameter("out", [TQ, E], F32, isOutput=True).ap()

    with tile.TileContext(nc) as tc:
        with (
            tc.tile_pool(name="persist", bufs=1) as pp,
            tc.tile_pool(name="pt", bufs=3) as ptp,
            tc.tile_pool(name="osb", bufs=2) as osbp,
            tc.tile_pool(name="onat", bufs=2) as onatp,
            tc.tile_pool(name="rz", bufs=2) as rzp,
            tc.tile_pool(name="vstage", bufs=4) as vsp,
            tc.tile_pool(name="outsb", bufs=2) as outp,
            tc.tile_pool(name="st", bufs=1, space="PSUM") as stp,
            tc.tile_pool(name="po", bufs=2, space="PSUM") as pop,
            tc.tile_pool(name="pj", bufs=2, space="PSUM") as pjp,
        ):
            # ---------- phase 0: loads ----------
            warm = pp.tile([1, 16], F32)
            nc.vector.memset(warm[:], 0.0)
            nc.scalar.activation(warm[:], warm[:], AF.Exp)

            # weights: tile [128, 2E]; slice e covers W rows [128e, 128e+128)
            w_sb = {}

            def load_w(name, wsrc):
                t = pp.tile([128, 2 * E], BF16, name=f"w{name}", tag=f"w{name}")
                nc.scalar.dma_start(
                    out=t[:].rearrange("p (e n) -> p e n", e=2),
                    in_=wsrc.rearrange("(e p) n -> p e n", p=128),
                )
                w_sb[name] = t

            load_w("k", wk)
            load_w("q", wq)

            # biases: bq/bk as per-partition columns [128, 2] (hc chunks);
            # bv/bp replicated across partitions
            bq_sb = pp.tile([128, 2], F32)
            nc.gpsimd.dma_start(
                out=bq_sb[:], in_=bq.rearrange("a (c p) -> p (a c)", p=128)
            )
            bk_sb = pp.tile([128, 2], F32)
            nc.gpsimd.dma_start(
                out=bk_sb[:], in_=bk.rearrange("a (c p) -> p (a c)", p=128)
            )
            bv_sb = pp.tile([128, E], F32)
            nc.gpsimd.dma_start(out=bv_sb[:], in_=bv.to_broadcast((128, E)))
            bp_sb = pp.tile([128, E], F32)
            nc.gpsimd.dma_start(out=bp_sb[:], in_=bp.to_broadcast((128, E)))

            # x.T loads AFTER weights (same HWDGE queue ordering): q first
            # (q-proj unblocks first), then k chunks in n order, then v.
            xq_sb = []
            for e in range(2):
                t = pp.tile([128, TQ], BF16, name=f"xqt{e}", tag=f"xqt{e}")
                nc.scalar.dma_start(out=t[:], in_=xqt[e * 128:(e + 1) * 128, :])
                xq_sb.append(t)
            xk_sb = [
                pp.tile([128, L], BF16, name=f"xkt{e}", tag=f"xkt{e}")
                for e in range(2)
            ]
            for n in range(2):
                for e in range(2):
                    nc.sync.dma_start(
                        out=xk_sb[e][:, n * 1024:(n + 1) * 1024],
                        in_=xkt[e * 128:(e + 1) * 128, n * 1024:(n + 1) * 1024],
                    )
            load_w("v", wv)
            load_w("p", wp)
            xv_sb = [
                pp.tile([128, L], BF16, name=f"xvt{e}", tag=f"xvt{e}")
                for e in range(2)
            ]
            for n in range(2):
                for e in range(2):
                    nc.sync.dma_start(
                        out=xv_sb[e][:, n * 1024:(n + 1) * 1024],
                        in_=xvt[e * 128:(e + 1) * 128, n * 1024:(n + 1) * 1024],
                    )

            # ---------- persistent SBUF state ----------
            # kT[hc]: [128 = 4 heads x 32 d (bands 0/32/64/96), 2048 tk]
            kT = [pp.tile([128, L], BF16, name=f"kT{hc}", tag=f"kT{hc}")
                  for hc in range(2)]
            qT = [pp.tile([128, TQ], BF16, name=f"qT{hc}", tag=f"qT{hc}")
                  for hc in range(2)]
            v_buf = pp.tile([128, NTK * VW], BF16)
            nc.gpsimd.memset(v_buf[:], 1.0)

            # score psum: 4 banks, one PER ROW GROUP (bank u <-> PE row
            # band 32u). Two units double-buffer via the column HALVES
            # of each bank (slot s = unit parity). Same-bank writers are
            # then always same-row-group matmuls, which the PE
            # serializes -- concurrent row-tiled matmuls to one psum
            # bank hang the device (HW-verified).
            st_all = stp.tile([128, 2048], F32, name="st_all")

            # ---------- q projection ----------
            for hc in range(2):
                ps = pjp.tile([128, TQ], F32, tag="pj")
                for e in range(2):
                    nc.tensor.matmul(
                        ps[:],
                        w_sb["q"][:, e * E + hc * 128: e * E + (hc + 1) * 128],
                        xq_sb[e][:, :],
                        start=(e == 0),
                        stop=(e == 1),
                    )
                nc.vector.tensor_scalar_add(
                    qT[hc][:, :], ps[:], bq_sb[:, hc:hc + 1]
                )

            # ---------- attention units ----------
            po_tiles = {}   # pass -> (poA, poB); poA: u 0/1, poB: u 2/3

            unit_seq = [0]

            def score_unit(p, g, j, use_dve):
                """scores + exp + PV for heads 4p..4p+3, tk chunk g,
                tq half j (256 cols)."""
                s = unit_seq[0] % 2
                unit_seq[0] += 1
                for u in (0, 2, 1, 3):
                    # each score MM is its own accumulation group; the
                    # start's whole-bank pending-zero mark only gates
                    # matmul WRITES (has_written bits), so the sibling
                    # half's data stays readable by the exp engines.
                    nc.tensor.matmul(
                        st_all[:, u * 512 + s * 256: u * 512 + s * 256 + 256],
                        kT[p][32 * u:32 * u + D, g * 128:(g + 1) * 128],
                        qT[p][32 * u:32 * u + D, j * 256:(j + 1) * 256],
                        start=True,
                        stop=True,
                        tile_position=(32 * u, 0),
                    )
                stv = st_all[:].rearrange("p (u w) -> p u w", u=4)[
                    :, :, s * 256:(s + 1) * 256
                ]
                pt = ptp.tile([128, 1024], BF16, tag="pt")
                ptv = pt[:].rearrange("p (u w) -> p u w", u=4)
                if use_dve:
                    # Schraudolph: bf16 bits of exp(s*SCALE) via one
                    # fused (x * C1) + C2 -> int16 convert.
                    nc.vector.tensor_scalar(
                        ptv.bitcast(I16), stv, C1, C2, ALU.mult, ALU.add
                    )
                else:
                    nc.scalar.activation(ptv, stv, AF.Exp, scale=SCALE)
                poA, poB = po_tiles[p]
                first = (g == 0 and j == 0)
                last = (g == NTK - 1 and j == 1)
                for u in range(4):
                    po = poA if u < 2 else poB
                    uu = u % 2
                    h = 4 * p + u
                    for m in range(2):
                        mg = 2 * j + m
                        nc.tensor.matmul(
                            po[:, uu * 132 + mg * 33: uu * 132 + mg * 33 + 33],
                            pt[:, u * 256 + m * 128: u * 256 + (m + 1) * 128],
                            v_buf[:, g * VW + h * (D + 1): g * VW + (h + 1) * (D + 1)],
                            start=(first and m == 0 and uu == 0),
                            stop=(last and m == 1 and uu == 1),
                            skip_group_check=True,
                        )

            # proj psum: two [128, 512] tiles hold the four [128, 256]
            # tq-chunk partials across both passes (allocated after the
            # last pj-pool ps allocation, see below)
            pjt = []

            onat_t = {}
            osb_t = {}
            rz_t = {}

            def finalize_half(p, j):
                """normalize + transpose + proj for tq half j of pass p."""
                poA, poB = po_tiles[p]
                if j == 0:
                    onat_t[p] = onatp.tile([128, TQ], BF16, name=f"onat{p}", tag="onat")
                    osb_t[p] = osbp.tile([128, TQ], BF16, name=f"osb{p}", tag="osb")
                    rz_t[p] = rzp.tile([128, 16], F32, name=f"rz{p}", tag="rz")
                onat, osb, rz = onat_t[p], osb_t[p], rz_t[p]
                # rz col layout: idx*8 + uu*4 + mg
                for idx, po in ((0, poA), (1, poB)):
                    # Z columns: po cols uu*132 + mg*33 + 32; this half's
                    # mg in {2j, 2j+1} -> [128, 2 (uu), 2 (mg), 1] strided
                    zv = po[:].rearrange("p (b m w) -> p b m w", b=2, m=4)[
                        :, :, 2 * j:2 * j + 2, D:D + 1
                    ]
                    rzo = rz[:, idx * 8:(idx + 1) * 8].rearrange(
                        "p (b m) -> p b m", b=2
                    )[:, :, 2 * j:2 * j + 2].unsqueeze(3)
                    nc.vector.reciprocal(rzo, zv)
                for u in range(4):
                    po = poA if u < 2 else poB
                    uu = u % 2
                    idx = u // 2
                    # in: po [128, 2 (mg of this half), 32] strided
                    pin = po[:].rearrange("p (mm w) -> p mm w", w=33)[
                        :, uu * 4 + 2 * j: uu * 4 + 2 * j + 2, 0:D
                    ]
                    rzb = rz[
                        :, idx * 8 + uu * 4 + 2 * j: idx * 8 + uu * 4 + 2 * j + 2
                    ].unsqueeze(2).to_broadcast((128, 2, D))
                    # out: onat cols m*128 + u*32, m in {2j, 2j+1}
                    pout = onat[:].rearrange(
                        "p (m b w) -> p m b w", m=4, b=4
                    )[:, 2 * j:2 * j + 2, u:u + 1, :]
                    nc.vector.tensor_tensor(pout, pin, rzb, ALU.mult)
                for m in (2 * j, 2 * j + 1):
                    eng = nc.sync if m % 2 == 0 else nc.scalar
                    eng.dma_start_transpose(
                        osb[:, m * 128:(m + 1) * 128],
                        onat[:, m * 128:(m + 1) * 128],
                    )
                    # start only on the FIRST chunk of each pjt bank:
                    # start=True marks the whole 2KB zero region, so a
                    # second start would wipe the sibling chunk's data.
                    nc.tensor.matmul(
                        pjt[m // 2][:, (m % 2) * E:(m % 2 + 1) * E],
                        osb[:, m * 128:(m + 1) * 128],
                        w_sb["p"][:, p * E:(p + 1) * E],
                        start=(p == 0 and m % 2 == 0),
                        stop=(p == NPASS - 1 and m % 2 == 1),
                        skip_group_check=True,
                    )

            # ---------- pass 0 (+ interleaved k/v projections) ----------
            po_tiles[0] = (
                pop.tile([128, 264], F32, name="poA", tag="po"),
                pop.tile([128, 264], F32, name="poB", tag="po"),
            )
            unit_idx = 0
            for n in range(4):
                # k projection for tk cols [512n, 512n+512)
                for hc in range(2):
                    ps = pjp.tile([128, 512], F32, tag="pj")
                    for e in range(2):
                        nc.tensor.matmul(
                            ps[:],
                            w_sb["k"][:, e * E + hc * 128: e * E + (hc + 1) * 128],
                            xk_sb[e][:, n * 512:(n + 1) * 512],
                            start=(e == 0),
                            stop=(e == 1),
                        )
                    nc.vector.tensor_scalar_add(
                        kT[hc][:, n * 512:(n + 1) * 512], ps[:],
                        bk_sb[:, hc:hc + 1],
                    )
                # v projection for tk chunks 4n..4n+3
                for t in range(4 * n, 4 * n + 4):
                    ps = pjp.tile([128, E], F32, tag="pj")
                    for e in range(2):
                        nc.tensor.matmul(
                            ps[:],
                            xv_sb[e][:, t * 128:(t + 1) * 128],
                            w_sb["v"][:, e * E:(e + 1) * E],
                            start=(e == 0),
                            stop=(e == 1),
                        )
                    vs = vsp.tile([128, E], BF16, tag="vstage")
                    nc.vector.tensor_tensor(vs[:], ps[:], bv_sb[:], ALU.add)
                    nc.sync.dma_start(
                        out=v_buf[:, t * VW:(t + 1) * VW].rearrange(
                            "p (h w) -> p h w", h=H
                        )[:, :, 0:D],
                        in_=vs[:].rearrange("p (h d) -> p h d", h=H),
                    )
                # pass-0 attention for tk chunks 4n..4n+3
                for g in range(4 * n, 4 * n + 4):
                    for j in range(2):
                        score_unit(0, g, j, (unit_idx % 8) in DVE_P0)
                        unit_idx += 1

            # pjt allocated after the final pj-pool ps allocation so the
            # round-robin slots are free to persist from here on
            for i in range(2):
                pjt.append(
                    pjp.tile([128, 2 * E], F32, name=f"pjt{i}", tag="pj")
                )
            finalize_half(0, 0)
            finalize_half(0, 1)

            # ---------- pass 1 ----------
            po_tiles[1] = (
                pop.tile([128, 264], F32, name="poA", tag="po"),
                pop.tile([128, 264], F32, name="poB", tag="po"),
            )
            for j in range(2):
                for g in range(NTK):
                    score_unit(1, g, j, (j * 16 + g) in DVE_P1)
                finalize_half(1, j)

            # ---------- out: bias + DMA ----------
            for m in range(TQ // 128):
                ob = outp.tile([128, E], F32, tag="outsb")
                nc.vector.tensor_tensor(
                    ob[:], pjt[m // 2][:, (m % 2) * E:(m % 2 + 1) * E],
                    bp_sb[:], ALU.add,
                )
                eng = nc.sync if m % 2 == 0 else nc.scalar
                eng.dma_start(
                    out=out[m * 128:(m + 1) * 128, :], in_=ob[:]
                )

    return nc


def get_graph():
    global _GRAPH
    if _GRAPH is None:
        nc = _build_graph()
        nc.compile()
        _GRAPH = nc
    return _GRAPH


def make_in_maps(query, key_, value, Wq, bq, Wk, bk, Wv, bv, Wp, bp):
    query = np.asarray(query, np.float32)
    key_ = np.asarray(key_, np.float32)
    value = np.asarray(value, np.float32)
    Wq, Wk, Wv, Wp = (np.asarray(w, np.float32) for w in (Wq, Wk, Wv, Wp))
    bq, bk, bv, bp = (np.asarray(b_, np.float32) for b_ in (bq, bk, bv, bp))

    wq_b = np.ascontiguousarray(Wq).astype(BF)
    wk_b = np.ascontiguousarray(Wk).astype(BF)
    wv_b = np.ascontiguousarray(Wv).astype(BF)
    wp_b = np.ascontiguousarray(Wp).astype(BF)
    xt = {}
    for b in range(B):
        xt[("q", b)] = np.ascontiguousarray(query[:, b, :].T).astype(BF)
        xt[("k", b)] = np.ascontiguousarray(key_[:, b, :].T).astype(BF)
        xt[("v", b)] = np.ascontiguousarray(value[:, b, :].T).astype(BF)

    in_maps = []
    for c in range(NCORES):
        b = c // SP
        p = c % SP
        m = {
            "xqt": np.ascontiguousarray(xt[("q", b)][:, p * TQ:(p + 1) * TQ]),
            "xkt": xt[("k", b)],
            "xvt": xt[("v", b)],
            "wq": wq_b,
            "wk": wk_b,
            "wv": wv_b,
            "wp": wp_b,
            "bq": bq.reshape(1, E).copy(),
            "bk": bk.reshape(1, E).copy(),
            "bv": bv.reshape(1, E).copy(),
            "bp": bp.reshape(1, E).copy(),
        }
        in_maps.append(m)
    return in_maps


def assemble(results):
    out_full = np.empty((L, B, E), np.float32)
    for c in range(NCORES):
        b = c // SP
        p = c % SP
        out_full[p * TQ:(p + 1) * TQ, b, :] = results[c]["out"]
    return out_full


def run(inputs, trace=False, **kw):
    nc = get_graph()
    in_maps = make_in_maps(**inputs)
    res = run_bass_kernel_spmd(
        nc, in_maps, core_ids=list(range(NCORES)), trace=trace, **kw
    )
    return res


def kernel(**inputs):
    res = run(inputs, trace=False)
    return assemble(res.results)


# revision 11
# speedup vs baseline: 1.1105x; 1.1105x over previous
"""Distributed attention kernel for 8 TRN2 NeuronCores.

Problem: L=2048, B=2, E=256, H=8 heads, D=32 head-dim, fp32.

Sharding: DP2 over batch x sequence-parallel-4 over query positions.
Core c handles batch c//4, query rows [512*(c%4), 512*(c%4+1)), ALL 8
heads. k/v projections are redundantly computed per batch group (cheap)
and NO collective is needed: each core owns a disjoint output block.

Per-core pipeline (v2 -- row-tiled scores + dual-engine exp):
  - kT/qT are stored with 4 heads stacked on partition bands 0/32/64/96
    so the K=32 score matmuls for 4 heads run CONCURRENTLY in the PE
    array via tile_position=(32u, 0) row tiling (the 128x128 array is
    16 32x32 subarrays; row-tiled matmuls with disjoint row groups
    overlap almost perfectly).
  - softmax exp is split across TWO engines running in parallel:
    ScalarE does exact exp via the ACT LUT; VectorE (DVE) computes a
    Schraudolph-style exp2 approximation (one fused mult+add
    tensor_scalar producing the bf16 BIT PATTERN as int16). The
    approximation has ~1.8% rms relative error, softmax-normalized;
    only a minority of key-blocks use it so the output error stays
    ~1.3% (budget 2e-2).
  - k/v projections are interleaved into pass 0's attention loop so PE
    never idles; pass 1 finalizes per-tq-half to shorten the tail.
  - PV uses P.T chunks as the STATIONARY operand and [v|1] as moving,
    so O lands in natural [tq, d] orientation with the softmax
    denominator Z per-partition (reciprocal + broadcast multiply);
    xbar DMA transposes produce O.T for the projection off-PE.
"""

import math
import os
import sys

import numpy as np

for _p in ("/opt/trn_rl_repo",):
    if _p not in sys.path and os.path.isdir(_p):
        sys.path.insert(0, _p)

import ml_dtypes

import concourse.bass as bass
import concourse.bacc as bacc
import concourse.mybir as mybir
import concourse.tile as tile
from concourse.bass_utils import run_bass_kernel_spmd

dt = mybir.dt
F32 = dt.float32
BF16 = dt.bfloat16
I16 = dt.int16
AF = mybir.ActivationFunctionType
ALU = mybir.AluOpType
BF = ml_dtypes.bfloat16

L, B, E, H, D = 2048, 2, 256, 8, 32
SCALE = float(D) ** -0.5
NCORES = 8
SP = 4            # sequence-parallel ways
TQ = L // SP      # 512 query rows per core
NTK = L // 128    # 16 tk chunks
VW = H * (D + 1)  # v_buf cols per tk chunk: 8x [v_h | 1] = 264
NPASS = 2         # head passes (4 heads each)

# Schraudolph exp2: bf16 bits of exp(s*SCALE) ~= int16(s*C1 + C2).
# C1 = 128 * SCALE * log2(e); C2 = 128*(127 - c) + 0.5 (c = rms-optimal
# 0.0434, +0.5 assumes truncating convert; the c offset is uniform
# across a softmax row so only the intra-octave wiggle matters).
C1 = 128.0 * SCALE * math.log2(math.e)
C2 = 128.0 * (127.0 - 0.0434) + 0.5

# exp engine assignment (True -> DVE Schraudolph, False -> ScalarE exp).
# pass 0 has 8 units per n-group (idx = (g%4)*2 + j); pass 1 has 32
# units (idx = j*16 + g).
DVE_P0 = (1, 4, 6)
DVE_P1 = tuple(i for i in range(32) if i % 2 == 1 and i not in (1, 17))

_GRAPH = None


def _build_graph():
    nc = bacc.Bacc(
        "TRN2",
        target_bir_lowering=False,
        debug=False,
        enable_asserts=False,
        num_devices=NCORES,
    )

    xqt = nc.declare_dram_parameter("xqt", [E, TQ], BF16, isOutput=False).ap()
    xkt = nc.declare_dram_parameter("xkt", [E, L], BF16, isOutput=False).ap()
    xvt = nc.declare_dram_parameter("xvt", [E, L], BF16, isOutput=False).ap()
    wq = nc.declare_dram_parameter("wq", [E, E], BF16, isOutput=False).ap()
    wk = nc.declare_dram_parameter("wk", [E, E], BF16, isOutput=False).ap()
    wv = nc.declare_dram_parameter("wv", [E, E], BF16, isOutput=False).ap()
    wp = nc.declare_dram_parameter("wp", [E, E], BF16, isOutput=False).ap()
    bq = nc.declare_dram_parameter("bq", [1, E], F32, isOutput=False).ap()
    bk = nc.declare_dram_parameter("bk", [1, E], F32, isOutput=False).ap()
    bv = nc.declare_dram_parameter("bv", [1, E], F32, isOutput=False).ap()
    bp = nc.declare_dram_parameter("bp", [1, E], F32, isOutput=False).ap()
    out = nc.declare_dram_parameter("out", [TQ, E], F32, isOutput=True).ap()

    with tile.TileContext(nc) as tc:
        with (
            tc.tile_pool(name="persist", bufs=1) as pp,
            tc.tile_pool(name="pt", bufs=3) as ptp,
            tc.tile_pool(name="osb", bufs=2) as osbp,
            tc.tile_pool(name="onat", bufs=2) as onatp,
            tc.tile_pool(name="rz", bufs=2) as rzp,
            tc.tile_pool(name="vstage", bufs=4) as vsp,
            tc.tile_pool(name="outsb", bufs=2) as outp,
            tc.tile_pool(name="st", bufs=1, space="PSUM") as stp,
            tc.tile_pool(name="po", bufs=2, space="PSUM") as pop,
            tc.tile_pool(name="pj", bufs=2, space="PSUM") as pjp,
        ):
            # ---------- phase 0: loads ----------
            warm = pp.tile([1, 16], F32)
            nc.vector.memset(warm[:], 0.0)
            nc.scalar.activation(warm[:], warm[:], AF.Exp)

            # weights: tile [128, 2E]; slice e covers W rows [128e, 128e+128)
            w_sb = {}

            def load_w(name, wsrc):
                t = pp.tile([128, 2 * E], BF16, name=f"w{name}", tag=f"w{name}")
                nc.scalar.dma_start(
                    out=t[:].rearrange("p (e n) -> p e n", e=2),
                    in_=wsrc.rearrange("(e p) n -> p e n", p=128),
                )
                w_sb[name] = t

            load_w("k", wk)
            load_w("q", wq)

            # biases: bq/bk as per-partition columns [128, 2] (hc chunks);
            # bv/bp replicated across partitions
            bq_sb = pp.tile([128, 2], F32)
            nc.gpsimd.dma_start(
                out=bq_sb[:], in_=bq.rearrange("a (c p) -> p (a c)", p=128)
            )
            bk_sb = pp.tile([128, 2], F32)
            nc.gpsimd.dma_start(
                out=bk_sb[:], in_=bk.rearrange("a (c p) -> p (a c)", p=128)
            )
            bv_sb = pp.tile([128, E], F32)
            nc.gpsimd.dma_start(out=bv_sb[:], in_=bv.to_broadcast((128, E)))
            bp_sb = pp.tile([128, E], F32)
            nc.gpsimd.dma_start(out=bp_sb[:], in_=bp.to_broadcast((128, E)))

            # x.T loads AFTER weights (same HWDGE queue ordering): q first
            # (q-proj unblocks first), then k chunks in n order, then v.
            xq_sb = []
            for e in range(2):
                t = pp.tile([128, TQ], BF16, name=f"xqt{e}", tag=f"xqt{e}")
                nc.scalar.dma_start(out=t[:], in_=xqt[e * 128:(e + 1) * 128, :])
                xq_sb.append(t)
            xk_sb = [
                pp.tile([128, L], BF16, name=f"xkt{e}", tag=f"xkt{e}")
                for e in range(2)
            ]
            for n in range(2):
                for e in range(2):
                    nc.sync.dma_start(
                        out=xk_sb[e][:, n * 1024:(n + 1) * 1024],
                        in_=xkt[e * 128:(e + 1) * 128, n * 1024:(n + 1) * 1024],
                    )
            load_w("v", wv)
            load_w("p", wp)
            xv_sb = [
                pp.tile([128, L], BF16, name=f"xvt{e}", tag=f"xvt{e}")
                for e in range(2)
            ]
            for n in range(2):
                for e in range(2):
                    nc.sync.dma_start(
                        out=xv_sb[e][:, n * 1024:(n + 1) * 1024],
                        in_=xvt[e * 128:(e + 1) * 128, n * 1024:(n + 1) * 1024],
                    )

            # ---------- persistent SBUF state ----------
            # kT[hc]: [128 = 4 heads x 32 d (bands 0/32/64/96), 2048 tk]
            kT = [pp.tile([128, L], BF16, name=f"kT{hc}", tag=f"kT{hc}")
                  for hc in range(2)]
            qT = [pp.tile([128, TQ], BF16, name=f"qT{hc}", tag=f"qT{hc}")
                  for hc in range(2)]
            v_buf = pp.tile([128, NTK * VW], BF16)
            nc.gpsimd.memset(v_buf[:], 1.0)

            # score psum: 4 banks, one PER ROW GROUP (bank u <-> PE row
            # band 32u). Two units double-buffer via the column HALVES
            # of each bank (slot s = unit parity). Same-bank writers are
            # then always same-row-group matmuls, which the PE
            # serializes -- concurrent row-tiled matmuls to one psum
            # bank hang the device (HW-verified).
            st_all = stp.tile([128, 2048], F32, name="st_all")

            # ---------- q projection ----------
            for hc in range(2):
                ps = pjp.tile([128, TQ], F32, tag="pj")
                for e in range(2):
                    nc.tensor.matmul(
                        ps[:],
                        w_sb["q"][:, e * E + hc * 128: e * E + (hc + 1) * 128],
                        xq_sb[e][:, :],
                        start=(e == 0),
                        stop=(e == 1),
                    )
                nc.vector.tensor_scalar_add(
                    qT[hc][:, :], ps[:], bq_sb[:, hc:hc + 1]
                )

            # ---------- attention units ----------
            po_tiles = {}   # pass -> (poA, poB); poA: u 0/1, poB: u 2/3

            unit_seq = [0]
            pv_pending = []   # software pipeline: PV runs one unit late

            def emit_pv(desc):
                """PV matmuls for an earlier unit (pt already exp'd)."""
                p, g, j, pt, first, last = desc
                poA, poB = po_tiles[p]
                for u in range(4):
                    po = poA if u < 2 else poB
                    uu = u % 2
                    h = 4 * p + u
                    for m in range(2):
                        mg = 2 * j + m
                        nc.tensor.matmul(
                            po[:, uu * 132 + mg * 33: uu * 132 + mg * 33 + 33],
                            pt[:, u * 256 + m * 128: u * 256 + (m + 1) * 128],
                            v_buf[:, g * VW + h * (D + 1): g * VW + (h + 1) * (D + 1)],
                            start=(first and m == 0 and uu == 0),
                            stop=(last and m == 1 and uu == 1),
                            skip_group_check=True,
                        )

            def flush_pv():
                while pv_pending:
                    emit_pv(pv_pending.pop(0))

            def score_unit(p, g, j, use_dve):
                """scores + exp for heads 4p..4p+3, tk chunk g, tq half
                j (256 cols); queues PV for one unit later so the next
                unit's scores sit ahead of the exp-blocked PV in the PE
                queue (the PE is in-order -- a PV waiting on exp would
                head-of-line block the next scores)."""
                s = unit_seq[0] % 2
                unit_seq[0] += 1
                for u in (0, 2, 1, 3):
                    # each score MM is its own accumulation group; the
                    # start's whole-bank pending-zero mark only gates
                    # matmul WRITES (has_written bits), so the sibling
                    # half's data stays readable by the exp engines.
                    nc.tensor.matmul(
                        st_all[:, u * 512 + s * 256: u * 512 + s * 256 + 256],
                        kT[p][32 * u:32 * u + D, g * 128:(g + 1) * 128],
                        qT[p][32 * u:32 * u + D, j * 256:(j + 1) * 256],
                        start=True,
                        stop=True,
                        tile_position=(32 * u, 0),
                    )
                stv = st_all[:].rearrange("p (u w) -> p u w", u=4)[
                    :, :, s * 256:(s + 1) * 256
                ]
                pt = ptp.tile([128, 1024], BF16, tag="pt")
                ptv = pt[:].rearrange("p (u w) -> p u w", u=4)
                if use_dve:
                    # Schraudolph: bf16 bits of exp(x*SCALE) via one
                    # fused (x * C1) + C2 -> int16 convert.
                    nc.vector.tensor_scalar(
                        ptv.bitcast(I16), stv, C1, C2, ALU.mult, ALU.add
                    )
                else:
                    nc.scalar.activation(ptv, stv, AF.Exp, scale=SCALE)
                first = (g == 0 and j == 0)
                last = (g == NTK - 1 and j == 1)
                desc = (p, g, j, pt, first, last)
                pv_pending.append(desc)
                while len(pv_pending) > 1:
                    emit_pv(pv_pending.pop(0))

            # proj psum: two [128, 512] tiles hold the four [128, 256]
            # tq-chunk partials across both passes (allocated after the
            # last pj-pool ps allocation, see below)
            pjt = []

            onat_t = {}
            osb_t = {}
            rz_t = {}

            def finalize_half(p, j):
                """normalize + transpose + proj for tq half j of pass p."""
                poA, poB = po_tiles[p]
                if j == 0:
                    onat_t[p] = onatp.tile([128, TQ], BF16, name=f"onat{p}", tag="onat")
                    osb_t[p] = osbp.tile([128, TQ], BF16, name=f"osb{p}", tag="osb")
                    rz_t[p] = rzp.tile([128, 16], F32, name=f"rz{p}", tag="rz")
                onat, osb, rz = onat_t[p], osb_t[p], rz_t[p]
                # rz col layout: idx*8 + uu*4 + mg
                for idx, po in ((0, poA), (1, poB)):
                    # Z columns: po cols uu*132 + mg*33 + 32; this half's
                    # mg in {2j, 2j+1} -> [128, 2 (uu), 2 (mg), 1] strided
                    zv = po[:].rearrange("p (b m w) -> p b m w", b=2, m=4)[
                        :, :, 2 * j:2 * j + 2, D:D + 1
                    ]
                    rzo = rz[:, idx * 8:(idx + 1) * 8].rearrange(
                        "p (b m) -> p b m", b=2
                    )[:, :, 2 * j:2 * j + 2].unsqueeze(3)
                    nc.vector.reciprocal(rzo, zv)
                for u in range(4):
                    po = poA if u < 2 else poB
                    uu = u % 2
                    idx = u // 2
                    # in: po [128, 2 (mg of this half), 32] strided
                    pin = po[:].rearrange("p (mm w) -> p mm w", w=33)[
                        :, uu * 4 + 2 * j: uu * 4 + 2 * j + 2, 0:D
                    ]
                    rzb = rz[
                        :, idx * 8 + uu * 4 + 2 * j: idx * 8 + uu * 4 + 2 * j + 2
                    ].unsqueeze(2).to_broadcast((128, 2, D))
                    # out: onat cols m*128 + u*32, m in {2j, 2j+1}
                    pout = onat[:].rearrange(
                        "p (m b w) -> p m b w", m=4, b=4
                    )[:, 2 * j:2 * j + 2, u:u + 1, :]
                    nc.vector.tensor_tensor(pout, pin, rzb, ALU.mult)
                for m in (2 * j, 2 * j + 1):
                    eng = nc.sync
                    eng.dma_start_transpose(
                        osb[:, m * 128:(m + 1) * 128],
                        onat[:, m * 128:(m + 1) * 128],
                    )
                    # start only on the FIRST chunk of each pjt bank:
                    # start=True marks the whole 2KB zero region, so a
                    # second start would wipe the sibling chunk's data.
                    nc.tensor.matmul(
                        pjt[m // 2][:, (m % 2) * E:(m % 2 + 1) * E],
                        osb[:, m * 128:(m + 1) * 128],
                        w_sb["p"][:, p * E:(p + 1) * E],
                        start=(p == 0 and m % 2 == 0),
                        stop=(p == NPASS - 1 and m % 2 == 1),
                        skip_group_check=True,
                    )

            # ---------- pass 0 (+ interleaved k/v projections) ----------
            po_tiles[0] = (
                pop.tile([128, 264], F32, name="poA", tag="po"),
                pop.tile([128, 264], F32, name="poB", tag="po"),
            )
            unit_idx = 0
            for n in range(4):
                # k projection for tk cols [512n, 512n+512)
                for hc in range(2):
                    ps = pjp.tile([128, 512], F32, tag="pj")
                    for e in range(2):
                        nc.tensor.matmul(
                            ps[:],
                            w_sb["k"][:, e * E + hc * 128: e * E + (hc + 1) * 128],
                            xk_sb[e][:, n * 512:(n + 1) * 512],
                            start=(e == 0),
                            stop=(e == 1),
                        )
                    nc.vector.tensor_scalar_add(
                        kT[hc][:, n * 512:(n + 1) * 512], ps[:],
                        bk_sb[:, hc:hc + 1],
                    )
                # v projection for tk chunks 4n..4n+3
                for t in range(4 * n, 4 * n + 4):
                    ps = pjp.tile([128, E], F32, tag="pj")
                    for e in range(2):
                        nc.tensor.matmul(
                            ps[:],
                            xv_sb[e][:, t * 128:(t + 1) * 128],
                            w_sb["v"][:, e * E:(e + 1) * E],
                            start=(e == 0),
                            stop=(e == 1),
                        )
                    vs = vsp.tile([128, E], BF16, tag="vstage")
                    nc.vector.tensor_tensor(vs[:], ps[:], bv_sb[:], ALU.add)
                    nc.sync.dma_start(
                        out=v_buf[:, t * VW:(t + 1) * VW].rearrange(
                            "p (h w) -> p h w", h=H
                        )[:, :, 0:D],
                        in_=vs[:].rearrange("p (h d) -> p h d", h=H),
                    )
                # pass-0 attention for tk chunks 4n..4n+3
                for g in range(4 * n, 4 * n + 4):
                    for j in range(2):
                        score_unit(0, g, j, (unit_idx % 8) in DVE_P0)
                        unit_idx += 1

            # pjt allocated after the final pj-pool ps allocation so the
            # round-robin slots are free to persist from here on
            for i in range(2):
                pjt.append(
                    pjp.tile([128, 2 * E], F32, name=f"pjt{i}", tag="pj")
                )
            flush_pv()
            finalize_half(0, 0)
            finalize_half(0, 1)

            # ---------- pass 1 ----------
            po_tiles[1] = (
                pop.tile([128, 264], F32, name="poA", tag="po"),
                pop.tile([128, 264], F32, name="poB", tag="po"),
            )
            for j in range(2):
                for g in range(NTK):
                    score_unit(1, g, j, (j * 16 + g) in DVE_P1)
                flush_pv()
                finalize_half(1, j)

            # ---------- out: bias + DMA ----------
            for m in range(TQ // 128):
                ob = outp.tile([128, E], F32, tag="outsb")
                nc.vector.tensor_tensor(
                    ob[:], pjt[m // 2][:, (m % 2) * E:(m % 2 + 1) * E],
                    bp_sb[:], ALU.add,
                )
                eng = nc.sync if m % 2 == 0 else nc.scalar
                eng.dma_start(
                    out=out[m * 128:(m + 1) * 128, :], in_=ob[:]
                )

    return nc


def get_graph():
    global _GRAPH
    if _GRAPH is None:
        nc = _build_graph()
        nc.compile()
        _GRAPH = nc
    return _GRAPH


def make_in_maps(query, key_, value, Wq, bq, Wk, bk, Wv, bv, Wp, bp):
    query = np.asarray(query, np.float32)
    key_ = np.asarray(key_, np.float32)
    value = np.asarray(value, np.float32)
    Wq, Wk, Wv, Wp = (np.asarray(w, np.float32) for w in (Wq, Wk, Wv, Wp))
    bq, bk, bv, bp = (np.asarray(b_, np.float32) for b_ in (bq, bk, bv, bp))

    wq_b = np.ascontiguousarray(Wq).astype(BF)
    wk_b = np.ascontiguousarray(Wk).astype(BF)
    wv_b = np.ascontiguousarray(Wv).astype(BF)
    wp_b = np.ascontiguousarray(Wp).astype(BF)
    xt = {}
    for b in range(B):
        xt[("q", b)] = np.ascontiguousarray(query[:, b, :].T).astype(BF)
        xt[("k", b)] = np.ascontiguousarray(key_[:, b, :].T).astype(BF)
        xt[("v", b)] = np.ascontiguousarray(value[:, b, :].T).astype(BF)

    in_maps = []
    for c in range(NCORES):
        b = c // SP
        p = c % SP
        m = {
            "xqt": np.ascontiguousarray(xt[("q", b)][:, p * TQ:(p + 1) * TQ]),
            "xkt": xt[("k", b)],
            "xvt": xt[("v", b)],
            "wq": wq_b,
            "wk": wk_b,
            "wv": wv_b,
            "wp": wp_b,
            "bq": bq.reshape(1, E).copy(),
            "bk": bk.reshape(1, E).copy(),
            "bv": bv.reshape(1, E).copy(),
            "bp": bp.reshape(1, E).copy(),
        }
        in_maps.append(m)
    return in_maps


def assemble(results):
    out_full = np.empty((L, B, E), np.float32)
    for c in range(NCORES):
        b = c // SP
        p = c % SP
        out_full[p * TQ:(p + 1) * TQ, b, :] = results[c]["out"]
    return out_full


def run(inputs, trace=False, **kw):
    nc = get_graph()
    in_maps = make_in_maps(**inputs)
    res = run_bass_kernel_spmd(
        nc, in_maps, core_ids=list(range(NCORES)), trace=trace, **kw
    )
    return res


def kernel(**inputs):
    res = run(inputs, trace=False)
    return assemble(res.results)
